# revision 22
# baseline (speedup 1.0000x reference)
"""Trainium2 Bass kernel for per-position head-attention (nn_DariushFlashAttention2).

Math (per batch b, sequence position s):
    Q = q[b,s].reshape(H=32, D=128); K, V likewise
    logits = Q @ K.T / sqrt(D)          # [32, 32] attention over HEADS
    W = softmax(logits, axis=-1)
    out[b,s] = (W @ V).reshape(H*D)

Every one of the B*S = 8192 positions is independent, so we shard positions
across the 8 NeuronCores (1024 positions each) and run one SPMD program.

Device strategy (per core), v7:
  - Positions packed 4-per-group onto the 128 SBUF partitions; host
    pre-transposes q,k into [d, (g,i,h)] fp16 and packs v as [(i,gh), (g,d|1)]
    with a ones-column per group; q|k|v concatenated so each 64-position
    chunk is ONE ~1.5 MB input DMA (16 total).
  - The Tile framework has only 8 HWDGE DMA semaphore lanes shared by every
    sync/scalar-ring DMA, so outputs drain through GPSIMD/SWDGE DMAs (own
    lane pool, otherwise-idle engine) and the HWDGE lanes belong exclusively
    to input prefetch.  in_pool is 10 deep so buffer releases stay well
    ahead of the wire (the stream runs at the mixed-r/w HBM roofline).
  - QK: per position one col-tiled matmul into a full-bank [128,512] PSUM
    tile per chunk; exp() runs per QUAD [128,128] so the WV matmuls'
    dependencies are satisfied before the in-order tensor engine reaches
    them (its wait-queue is only 4 deep — a parked WV blocks everything).
  - WV: per position a (32j,32j) sub-array matmul; the ones-column makes the
    same matmul emit the softmax denominator in its last column.
  - PSUM is evacuated with plain wide [128,258] copies (no reciprocal): the
    unnormalized outputs AND denominators ship to the host, which divides.
"""

import numpy as np

B, S, H, D = 2, 4096, 32, 128
NCORES = 8
POS = B * S                  # 8192 positions total
PPC = POS // NCORES          # 1024 positions per core
GP = 4                       # positions per group (4*32 heads = 128 partitions)
NG = 16                      # groups per chunk
CHUNK_POS = GP * NG          # 64 positions per chunk
NCHUNK = PPC // CHUNK_POS    # 16 chunks per core
VCOL = D + 1                 # v columns per group incl. ones column

QK_COLS = NG * D             # 2048
IN_COLS = 2 * QK_COLS + NG * VCOL   # q | k | v  = 6160
V_OFF = 2 * QK_COLS          # 4096
OUT_COLS = NG * VCOL         # 2064 = 8 pairs x 258

_SCALE = float(1.0 / np.sqrt(D))

_program = None  # cached compiled Bass program


def _build_program():
    import concourse.bacc as bacc
    import concourse.mybir as mybir
    from concourse.tile import TileContext

    fp32 = mybir.dt.float32
    fp16 = mybir.dt.float16

    nc = bacc.Bacc()
    qkv = nc.dram_tensor("qkv", [NCHUNK, 128, IN_COLS], fp16, kind="ExternalInput")
    out = nc.dram_tensor("out", [NCHUNK, 128, OUT_COLS], fp16, kind="ExternalOutput")

    with TileContext(nc) as tc:
        with (
            tc.tile_pool(name="qkv_in", bufs=12) as in_pool,
            tc.tile_pool(name="o_out", bufs=4) as o_pool,
            tc.tile_pool(name="exp", bufs=4) as exp_pool,
            tc.tile_pool(name="psl", bufs=3, space="PSUM") as psl_pool,
            tc.tile_pool(name="pso", bufs=5, space="PSUM") as pso_pool,
        ):
            # Inputs alternate between the two HWDGE rings (sync=SP and
            # scalar=ACT descriptor generators) so two transfers stream
            # concurrently — a single ring's descriptor generator caps a lone
            # transfer well below the HBM read rate.
            in_tiles = []

            def issue_in(n):
                in_t = in_pool.tile([128, IN_COLS], fp16, tag="qkv")
                eng = nc.sync if n % 2 == 0 else nc.scalar
                eng.dma_start(out=in_t, in_=qkv[n])
                in_tiles.append(in_t)

            for n in range(12):
                issue_in(n)

            # Chain every tensor instruction to the previous one so the
            # compiled tensor stream follows ISSUE order exactly — the
            # scheduler's sim otherwise rebuilds a data-lockstepped order
            # that parks the 4-deep wait queue on not-yet-ready WV work.
            _tlast = [None]

            def _chain(binst):
                inst = binst.ins if hasattr(binst, "ins") else binst
                if _tlast[0] is not None:
                    inst.add_dependency(_tlast[0], mybir.DependencyInfo.SYNC_ONLY)
                _tlast[0] = inst.name
                return binst

            def issue_qk(n):
                """Chunk n's logits into one full [128,512] PSUM bank."""
                in_t = in_tiles[n]
                psl = psl_pool.tile([128, 512], fp32, tag="psl")
                for g in range(NG):
                    q4, t = g >> 2, g & 3
                    for j in range(GP):
                        c = slice(g * D + 32 * j, g * D + 32 * j + 32)
                        ck = slice(QK_COLS + g * D + 32 * j,
                                   QK_COLS + g * D + 32 * j + 32)
                        _chain(nc.tensor.matmul(
                            psl[32 * j:32 * j + 32,
                                q4 * 128 + 32 * t:q4 * 128 + 32 * t + 32],
                            in_t[:, ck],       # stationary: k of (g, j)
                            in_t[:, c],        # moving:     q of (g, j)
                            start=True, stop=True,
                            tile_position=(0, 32 * j),
                        ))
                return psl

            def issue_exp(psl):
                # exp per quad (subtile deps: each waits only its quad's QK);
                # high priority so ready exps always beat copies/triggers in
                # the scalar engine's queue.
                exp_sb = exp_pool.tile([128, 512], fp16, tag="exp_sb")
                with tc.high_priority():
                    for q in range(4):
                        cs = slice(q * 128, (q + 1) * 128)
                        nc.scalar.activation(
                            exp_sb[:, cs], psl[:, cs],
                            mybir.ActivationFunctionType.Exp, scale=_SCALE,
                        )
                return exp_sb

            def issue_wv(n, exp_sb):
                """WV + denominator for chunk n, evacuate unnormalized."""
                in_t = in_tiles[n]
                out_t = o_pool.tile([128, OUT_COLS], fp16, tag="out")
                for pair in range(NG // 2):
                    psum_o = pso_pool.tile([128, 2 * VCOL], fp32, tag="pso")
                    for u in range(2):
                        g = 2 * pair + u
                        q4, t = g >> 2, g & 3
                        for j in range(GP):
                            r = slice(32 * j, 32 * j + 32)
                            _chain(nc.tensor.matmul(
                                psum_o[r, u * VCOL:(u + 1) * VCOL],
                                exp_sb[r, q4 * 128 + 32 * t:q4 * 128 + 32 * t + 32],
                                in_t[r, V_OFF + g * VCOL:V_OFF + (g + 1) * VCOL],
                                start=True, stop=True,
                                tile_position=(32 * j, 32 * j),
                            ))
                    dst = out_t[:, pair * 2 * VCOL:(pair + 1) * 2 * VCOL]
                    if pair == 0:
                        nc.scalar.copy(dst, psum_o)
                    else:
                        nc.vector.tensor_copy(dst, psum_o)

                # output drains via SWDGE: own sem lanes, idle engine
                nc.gpsimd.dma_start(out=out[n], in_=out_t)

            # Two-chunk-deep software pipeline: WV(n) is issued alongside
            # QK(n+2), so exp(n) / copies / out-buffer releases are all
            # ~2 chunks old (and long satisfied) by the time the in-order
            # tensor engine reaches a WV instruction.  No cross-engine wait
            # ever parks the 4-deep tensor wait-queue.
            exps = [issue_exp(issue_qk(0))]
            exps.append(issue_exp(issue_qk(1)))
            for n in range(NCHUNK):
                if n + 12 < NCHUNK:
                    issue_in(n + 12)
                if n + 2 < NCHUNK:
                    exps.append(issue_exp(issue_qk(n + 2)))
                issue_wv(n, exps[n])

    nc.compile()
    return nc


def _host_pack(q, k, v):
    """Build per-core device input arrays from full fp32 inputs."""
    qf = np.ascontiguousarray(q, dtype=np.float32).reshape(POS, H, D)
    kf = np.ascontiguousarray(k, dtype=np.float32).reshape(POS, H, D)
    vf = np.ascontiguousarray(v, dtype=np.float32).reshape(POS, H, D)

    nchunk_tot = POS // CHUNK_POS
    # q,k: [chunk, group, i, h, d] -> [chunk, d, (group, i, h)]
    def to_qt(x):
        x = x.reshape(nchunk_tot, NG, GP, H, D)
        x = x.transpose(0, 4, 1, 2, 3)
        return np.ascontiguousarray(x.reshape(nchunk_tot, D, NG * GP * H)).astype(np.float16)

    qt_all = to_qt(qf)
    kt_all = to_qt(kf)

    # v: [chunk, group, i, gh, d] -> [chunk, (i,gh), (group, d|1)]
    vv = vf.reshape(nchunk_tot, NG, GP, H, D).transpose(0, 2, 3, 1, 4)
    vp_all = np.ones((nchunk_tot, GP, H, NG, VCOL), dtype=np.float32)
    vp_all[..., :D] = vv
    vp_all = vp_all.reshape(nchunk_tot, GP * H, NG * VCOL).astype(np.float16)

    qkv_all = np.concatenate([qt_all, kt_all, vp_all], axis=2)
    qkv_all = np.ascontiguousarray(qkv_all)

    in_maps = []
    for c in range(NCORES):
        sl = slice(c * NCHUNK, (c + 1) * NCHUNK)
        in_maps.append({"qkv": np.ascontiguousarray(qkv_all[sl])})
    return in_maps


def _host_unpack(outs):
    """Per-core [NCHUNK, 128, NG*VCOL] fp16 -> full [B, S, H*D] fp32."""
    full = np.concatenate(outs, axis=0).astype(np.float32)
    nchunk_tot = POS // CHUNK_POS
    full = full.reshape(nchunk_tot, GP, H, NG, VCOL)  # [chunk, i, h, g, d|z]
    num = full[..., :D]
    den = full[..., D:D + 1]
    res = num / den
    res = res.transpose(0, 3, 1, 2, 4)                # [chunk, g, i, h, d]
    return np.ascontiguousarray(res.reshape(B, S, H * D), dtype=np.float32)


def kernel(q, k, v, _trace=False):
    global _program
    from concourse.bass_utils import run_bass_kernel_spmd

    if _program is None:
        _program = _build_program()

    in_maps = _host_pack(q, k, v)
    res = run_bass_kernel_spmd(_program, in_maps, list(range(NCORES)), trace=_trace)
    outs = [res.results[c]["out"] for c in range(NCORES)]
    result = _host_unpack(outs)
    if _trace:
        return result, res
    return result


# revision 23
# speedup vs baseline: 4.8042x; 4.8042x over previous
"""Trainium2 Bass kernel for per-position head-attention (nn_DariushFlashAttention2).

Math (per batch b, sequence position s):
    Q = q[b,s].reshape(H=32, D=128); K, V likewise
    logits = Q @ K.T / sqrt(D)          # [32, 32] attention over HEADS
    W = softmax(logits, axis=-1)
    out[b,s] = (W @ V).reshape(H*D)

Every one of the B*S = 8192 positions is independent, so we shard positions
across the 8 NeuronCores (1024 positions each) and run one SPMD program.

Device strategy (per core), v7:
  - Positions packed 4-per-group onto the 128 SBUF partitions; host
    pre-transposes q,k into [d, (g,i,h)] fp16 and packs v as [(i,gh), (g,d|1)]
    with a ones-column per group; q|k|v concatenated so each 64-position
    chunk is ONE ~1.5 MB input DMA (16 total).
  - The Tile framework has only 8 HWDGE DMA semaphore lanes shared by every
    sync/scalar-ring DMA, so outputs drain through GPSIMD/SWDGE DMAs (own
    lane pool, otherwise-idle engine) and the HWDGE lanes belong exclusively
    to input prefetch.  in_pool is 10 deep so buffer releases stay well
    ahead of the wire (the stream runs at the mixed-r/w HBM roofline).
  - QK: per position one col-tiled matmul into a full-bank [128,512] PSUM
    tile per chunk; exp() runs per QUAD [128,128] so the WV matmuls'
    dependencies are satisfied before the in-order tensor engine reaches
    them (its wait-queue is only 4 deep — a parked WV blocks everything).
  - WV: per position a (32j,32j) sub-array matmul; the ones-column makes the
    same matmul emit the softmax denominator in its last column.
  - PSUM is evacuated with plain wide [128,258] copies (no reciprocal): the
    unnormalized outputs AND denominators ship to the host, which divides.
"""

import numpy as np

B, S, H, D = 2, 4096, 32, 128
NCORES = 8
POS = B * S                  # 8192 positions total
PPC = POS // NCORES          # 1024 positions per core
GP = 4                       # positions per group (4*32 heads = 128 partitions)
NG = 16                      # groups per chunk
CHUNK_POS = GP * NG          # 64 positions per chunk
NCHUNK = PPC // CHUNK_POS    # 16 chunks per core
VCOL = D + 1                 # v columns per group incl. ones column

QK_COLS = NG * D             # 2048
IN_COLS = 2 * QK_COLS + NG * VCOL   # q | k | v  = 6160
V_OFF = 2 * QK_COLS          # 4096
OUT_COLS = NG * VCOL         # 2064 = 8 pairs x 258

_SCALE = float(1.0 / np.sqrt(D))

_program = None  # cached compiled Bass program


def _build_program():
    import concourse.bacc as bacc
    import concourse.mybir as mybir
    from concourse.tile import TileContext

    fp32 = mybir.dt.float32
    fp16 = mybir.dt.float16

    nc = bacc.Bacc()
    qkv = nc.dram_tensor("qkv", [NCHUNK, 128, IN_COLS], fp16, kind="ExternalInput")
    out = nc.dram_tensor("out", [NCHUNK, 128, OUT_COLS], fp16, kind="ExternalOutput")

    with TileContext(nc) as tc:
        with (
            tc.tile_pool(name="qkv_in", bufs=12) as in_pool,
            tc.tile_pool(name="o_out", bufs=4) as o_pool,
            tc.tile_pool(name="exp", bufs=4) as exp_pool,
            tc.tile_pool(name="psl", bufs=3, space="PSUM") as psl_pool,
            tc.tile_pool(name="pso", bufs=5, space="PSUM") as pso_pool,
        ):
            # Inputs alternate between the two HWDGE rings (sync=SP and
            # scalar=ACT descriptor generators) so two transfers stream
            # concurrently — a single ring's descriptor generator caps a lone
            # transfer well below the HBM read rate.
            in_tiles = []

            def issue_in(n):
                in_t = in_pool.tile([128, IN_COLS], fp16, tag="qkv")
                eng = nc.sync if n % 2 == 0 else nc.scalar
                eng.dma_start(out=in_t, in_=qkv[n])
                in_tiles.append(in_t)

            for n in range(12):
                issue_in(n)

            # Chain every tensor instruction to the previous one so the
            # compiled tensor stream follows ISSUE order exactly — the
            # scheduler's sim otherwise rebuilds a data-lockstepped order
            # that parks the 4-deep wait queue on not-yet-ready WV work.
            _tlast = [None]

            def _chain(binst):
                inst = binst.ins if hasattr(binst, "ins") else binst
                if _tlast[0] is not None:
                    inst.add_dependency(_tlast[0], mybir.DependencyInfo.NO_SYNC_ONLY)
                _tlast[0] = inst.name
                return binst

            def issue_qk(n):
                """Chunk n's logits into one full [128,512] PSUM bank."""
                in_t = in_tiles[n]
                psl = psl_pool.tile([128, 512], fp32, tag="psl")
                for g in range(NG):
                    q4, t = g >> 2, g & 3
                    for j in range(GP):
                        c = slice(g * D + 32 * j, g * D + 32 * j + 32)
                        ck = slice(QK_COLS + g * D + 32 * j,
                                   QK_COLS + g * D + 32 * j + 32)
                        _chain(nc.tensor.matmul(
                            psl[32 * j:32 * j + 32,
                                q4 * 128 + 32 * t:q4 * 128 + 32 * t + 32],
                            in_t[:, ck],       # stationary: k of (g, j)
                            in_t[:, c],        # moving:     q of (g, j)
                            start=True, stop=True,
                            tile_position=(0, 32 * j),
                        ))
                return psl

            def issue_exp(psl):
                # exp per quad (subtile deps: each waits only its quad's QK);
                # high priority so ready exps always beat copies/triggers in
                # the scalar engine's queue.
                exp_sb = exp_pool.tile([128, 512], fp16, tag="exp_sb")
                with tc.high_priority():
                    for q in range(4):
                        cs = slice(q * 128, (q + 1) * 128)
                        nc.scalar.activation(
                            exp_sb[:, cs], psl[:, cs],
                            mybir.ActivationFunctionType.Exp, scale=_SCALE,
                        )
                return exp_sb

            def issue_wv(n, exp_sb):
                """WV + denominator for chunk n, evacuate unnormalized."""
                in_t = in_tiles[n]
                out_t = o_pool.tile([128, OUT_COLS], fp16, tag="out")
                for pair in range(NG // 2):
                    psum_o = pso_pool.tile([128, 2 * VCOL], fp32, tag="pso")
                    for u in range(2):
                        g = 2 * pair + u
                        q4, t = g >> 2, g & 3
                        for j in range(GP):
                            r = slice(32 * j, 32 * j + 32)
                            _chain(nc.tensor.matmul(
                                psum_o[r, u * VCOL:(u + 1) * VCOL],
                                exp_sb[r, q4 * 128 + 32 * t:q4 * 128 + 32 * t + 32],
                                in_t[r, V_OFF + g * VCOL:V_OFF + (g + 1) * VCOL],
                                start=True, stop=True,
                                tile_position=(32 * j, 32 * j),
                            ))
                    dst = out_t[:, pair * 2 * VCOL:(pair + 1) * 2 * VCOL]
                    if pair == 0:
                        nc.scalar.copy(dst, psum_o)
                    else:
                        nc.vector.tensor_copy(dst, psum_o)

                # output drains via SWDGE: own sem lanes, idle engine
                nc.gpsimd.dma_start(out=out[n], in_=out_t)

            # Two-chunk-deep software pipeline: WV(n) is issued alongside
            # QK(n+2), so exp(n) / copies / out-buffer releases are all
            # ~2 chunks old (and long satisfied) by the time the in-order
            # tensor engine reaches a WV instruction.  No cross-engine wait
            # ever parks the 4-deep tensor wait-queue.
            exps = [issue_exp(issue_qk(0))]
            exps.append(issue_exp(issue_qk(1)))
            for n in range(NCHUNK):
                if n + 12 < NCHUNK:
                    issue_in(n + 12)
                if n + 2 < NCHUNK:
                    exps.append(issue_exp(issue_qk(n + 2)))
                issue_wv(n, exps[n])

    nc.compile()
    return nc


def _host_pack(q, k, v):
    """Build per-core device input arrays from full fp32 inputs."""
    qf = np.ascontiguousarray(q, dtype=np.float32).reshape(POS, H, D)
    kf = np.ascontiguousarray(k, dtype=np.float32).reshape(POS, H, D)
    vf = np.ascontiguousarray(v, dtype=np.float32).reshape(POS, H, D)

    nchunk_tot = POS // CHUNK_POS
    # q,k: [chunk, group, i, h, d] -> [chunk, d, (group, i, h)]
    def to_qt(x):
        x = x.reshape(nchunk_tot, NG, GP, H, D)
        x = x.transpose(0, 4, 1, 2, 3)
        return np.ascontiguousarray(x.reshape(nchunk_tot, D, NG * GP * H)).astype(np.float16)

    qt_all = to_qt(qf)
    kt_all = to_qt(kf)

    # v: [chunk, group, i, gh, d] -> [chunk, (i,gh), (group, d|1)]
    vv = vf.reshape(nchunk_tot, NG, GP, H, D).transpose(0, 2, 3, 1, 4)
    vp_all = np.ones((nchunk_tot, GP, H, NG, VCOL), dtype=np.float32)
    vp_all[..., :D] = vv
    vp_all = vp_all.reshape(nchunk_tot, GP * H, NG * VCOL).astype(np.float16)

    qkv_all = np.concatenate([qt_all, kt_all, vp_all], axis=2)
    qkv_all = np.ascontiguousarray(qkv_all)

    in_maps = []
    for c in range(NCORES):
        sl = slice(c * NCHUNK, (c + 1) * NCHUNK)
        in_maps.append({"qkv": np.ascontiguousarray(qkv_all[sl])})
    return in_maps


def _host_unpack(outs):
    """Per-core [NCHUNK, 128, NG*VCOL] fp16 -> full [B, S, H*D] fp32."""
    full = np.concatenate(outs, axis=0).astype(np.float32)
    nchunk_tot = POS // CHUNK_POS
    full = full.reshape(nchunk_tot, GP, H, NG, VCOL)  # [chunk, i, h, g, d|z]
    num = full[..., :D]
    den = full[..., D:D + 1]
    res = num / den
    res = res.transpose(0, 3, 1, 2, 4)                # [chunk, g, i, h, d]
    return np.ascontiguousarray(res.reshape(B, S, H * D), dtype=np.float32)


def kernel(q, k, v, _trace=False):
    global _program
    from concourse.bass_utils import run_bass_kernel_spmd

    if _program is None:
        _program = _build_program()

    in_maps = _host_pack(q, k, v)
    res = run_bass_kernel_spmd(_program, in_maps, list(range(NCORES)), trace=_trace)
    outs = [res.results[c]["out"] for c in range(NCORES)]
    result = _host_unpack(outs)
    if _trace:
        return result, res
    return result
